# revision 21
# baseline (speedup 1.0000x reference)
"""Bass kernel v3 for 2-layer LSTM encoder (B=128, T=512, D=128, H=512).

Architecture (8 cores, batch-quarter x layer split), same as v2:
  cores 0-3: layer-1 recurrence for batch quarter q=cid   (Bc=32)
  cores 4-7: layer-2 recurrence for batch quarter q=cid-4 (lag LAG chunks)
Pairs (q, q+4) exchange layer-1 output chunks via AllGather.  All cores
execute ONE uniform instruction stream (role differences are data only):
xg = wib@x + wob@out1 + bias with exactly one of wib/wob nonzero per
core.

v3 changes (the v2 profile showed the step period was bound by the
serial ACT/DVE chain, ~4.0us/step, not the matmul stream ~0.7us):
  * tanh eliminated via tanh(x) = 2*sigmoid(2x) - 1.  The g-gate rows of
    all weight/bias tensors are pre-scaled by 2 on the host, so ONE
    sigmoid instruction covers all four gates of a half-step.  The cell
    update becomes  c = f*c + 2*(sg_g - 0.5)*sg_i  (2 fused DVE ops +
    1 mul), and the hidden state is kept as  h2 = h/2 =
    (sigmoid(2c) - 0.5)*sg_o  (1 fused DVE op).  The missing 2x on h is
    folded into the CONSUMERS of h: whh columns and wob (=W_ih2)
    columns are pre-scaled by 2, and the final h output is scaled by 2
    on the host.  ACT: 6 instrs/step -> 4; DVE: ~8 plain -> 8 fused-or-
    cheap ops/step.
  * xg production runs one full chunk per sweep (SUB=16): one
    [128,512]-col matmul per (mi, source) instead of four, one bias add
    per mi per chunk.
  * step matmul order: inject xg first (off critical path), then j=0,1
    for all mi (needs only h half lo), then j=2,3 for mi 0..7 (half-lo
    gates complete -> chain starts), then j=2,3 for mi 8..15.

On-chip layout is "transposed": tiles are [128 partitions, free].
  h2   -> rolling SBUF tiles [P, (s, j, b)] fp16, SUB_H=4 steps per tile
  c    -> SBUF [P, (j, b)] fp32
  gates -> PSUM [P, (mi, b)] fp32, one bank per step
m-chunk order (host permutes weight rows): mi = 4*j + {i,f,o,g}.
"half lo" = j in {0,1} = mi in [0,8); "hi" = j in {2,3} = mi in [8,16).
"""
import sys
sys.path.insert(0, "/opt/trn_rl_repo")
import numpy as np
from concourse import bacc
import concourse.bass as bass
import concourse.mybir as mybir
import concourse.tile as tile

F16 = mybir.dt.float16
F32 = mybir.dt.float32
U32 = mybir.dt.uint32

N_CORES = 8
CORE_IDS = list(range(N_CORES))
PAIRS = [[0, 4], [1, 5], [2, 6], [3, 7]]
P = 128
B = 128
D = 128
H = 512
G4 = 2048
NJ = H // P          # 4
NM = G4 // P         # 16
Bc = B // 4          # 32 batch per core
F = NJ * Bc          # 128 state cols per core
CHUNK = 16           # steps per chunk
SUB_H = 4            # steps per rolling h tile (DMA granularity)

SIG = mybir.ActivationFunctionType.Sigmoid
ALU = mybir.AluOpType

LAG = 3  # L2 processes chunk k-LAG at iteration k


def build(nch):
    """nch = number of real chunks (T = nch*CHUNK); ITERS = nch+LAG."""
    iters = nch + LAG
    nc = bacc.Bacc()

    # ---------------- inputs ----------------
    xm_in = nc.declare_dram_parameter("x_my", [D, iters * CHUNK * Bc], F16, isOutput=False)
    whh_in = nc.declare_dram_parameter("whh", [P, NJ * NM * P], F16, isOutput=False)
    wib_in = nc.declare_dram_parameter("wib", [P, NM * P], F16, isOutput=False)
    wob_in = nc.declare_dram_parameter("wob", [P, NJ * NM * P], F16, isOutput=False)
    bias_in = nc.declare_dram_parameter("bias", [P, NM], F32, isOutput=False)
    bias0_in = nc.declare_dram_parameter("bias0", [P, NM], F32, isOutput=False)
    ident_in = nc.declare_dram_parameter("ident", [P, P], F16, isOutput=False)
    roles_in = nc.declare_dram_parameter("roles", [1, 3], U32, isOutput=False)

    # ---------------- outputs ----------------
    h_out = nc.declare_dram_parameter("h_out", [P, F], F32, isOutput=True)
    c_out = nc.declare_dram_parameter("c_out", [P, F], F32, isOutput=True)

    # ---------------- internal DRAM ----------------
    out1_stage = [nc.dram_tensor(f"o1s{k}", [P, CHUNK * F], F16) for k in range(iters)]
    # pairwise AllGather with a LOCAL (non-Shared) output: the ">4 cores"
    # restriction applies only to Shared-output collectives.  A 2-rank
    # gather moves 1/4 the bytes of the 8-wide one and, more importantly,
    # only synchronizes the two cores of a pair (the 8-wide gather acted
    # as a global barrier paying the full inter-core drift every chunk).
    # Index 0 of the gathered tensor = the L1 member of the pair.
    out1_full = [nc.dram_tensor(f"o1f{k}", [2, P, CHUNK * F], F16)
                 for k in range(iters)]

    with tile.TileContext(nc) as tc:
        with (
            tc.tile_pool(name="wpool", bufs=1) as wpool,
            tc.tile_pool(name="xgp", bufs=2) as xgp,
            tc.tile_pool(name="xsp", bufs=2) as xsp,
            tc.tile_pool(name="o1p", bufs=2) as o1p,
            tc.tile_pool(name="hrol", bufs=2) as hrol,
            tc.tile_pool(name="state", bufs=1) as state,
            tc.tile_pool(name="actp", bufs=2) as actp,
            tc.tile_pool(name="small", bufs=2) as small,
            tc.tile_pool(name="ps", bufs=2, space="PSUM") as psp,
            tc.tile_pool(name="pp", bufs=2, space="PSUM") as ppp,
        ):
            # ---- constants ----
            whh = wpool.tile([P, NJ * NM * P], F16)
            wib = wpool.tile([P, NM * P], F16)
            wob = wpool.tile([P, NJ * NM * P], F16)
            bias = wpool.tile([P, NM], F32)
            bias0 = wpool.tile([P, NM], F32)
            ident = wpool.tile([P, P], F16)
            zero1 = wpool.tile([1, P], F16)
            roles_t = wpool.tile([1, 3], U32)
            nc.vector.memset(zero1, 0.0)
            nc.sync.dma_start(out=whh, in_=whh_in[:, :])
            nc.sync.dma_start(out=wib, in_=wib_in[:, :])
            nc.sync.dma_start(out=wob, in_=wob_in[:, :])
            nc.sync.dma_start(out=bias, in_=bias_in[:, :])
            nc.sync.dma_start(out=bias0, in_=bias0_in[:, :])
            nc.sync.dma_start(out=ident, in_=ident_in[:, :])
            nc.sync.dma_start(out=roles_t, in_=roles_in[:, :])
            slot_reg = nc.scalar.alloc_register("slot_reg")
            nc.scalar.reg_load(slot_reg, roles_t[0:1, 2:3])
            slot = nc.scalar.snap(slot_reg, min_val=0, max_val=3)

            # ---- state ----
            cT = state.tile([P, NJ, Bc], F32)
            zero_h = state.tile([P, F], F16)
            nc.vector.memset(cT, 0.0)
            nc.vector.memset(zero_h, 0.0)

            # ---- one recurrence step ----
            # G layout: [P, mi(16), Bc]; mi = 4*j + {i,f,o,g}
            def step_mms(xg_s, h_lo, h_hi):
                """Emit the step's matmuls. h_lo/h_hi: [P, 2, Bc] fp16 views
                (j=0,1 and j=2,3 of the previous step's h2)."""
                Gt = psp.tile([P, NM * Bc], F32, tag="gates")
                nc.tensor.matmul(Gt, ident, xg_s, start=True, stop=False,
                                 skip_group_check=True)
                for j in range(2):
                    for mi in range(NM):
                        nc.tensor.matmul(
                            Gt[:, mi * Bc:(mi + 1) * Bc],
                            whh[:, (j * NM + mi) * P:(j * NM + mi + 1) * P],
                            h_lo[:, j, :],
                            start=False, stop=False, skip_group_check=True)
                for mh in range(2):
                    for mi in range(mh * 8, mh * 8 + 8):
                        for j in range(2, 4):
                            nc.tensor.matmul(
                                Gt[:, mi * Bc:(mi + 1) * Bc],
                                whh[:, (j * NM + mi) * P:(j * NM + mi + 1) * P],
                                h_hi[:, j - 2, :],
                                start=False, stop=(j == 3), skip_group_check=True)
                return Gt

            def chain_half(Gt, mh, h_new):
                """Elementwise chain for half mh (j in {2mh, 2mh+1}).
                h_new: [P, 2, Bc] fp16 view to write h2 = h/2 into."""
                Gh = Gt[:, mh * 8 * Bc:(mh + 1) * 8 * Bc].rearrange(
                    "p (j g b) -> p j g b", j=2, g=4)
                Sg = actp.tile([P, 2, 4, Bc], F32, tag=f"S{mh}")
                nc.scalar.activation(Sg, Gh, SIG)
                cs = cT[:, 2 * mh:2 * mh + 2, :]
                m = small.tile([P, 2, Bc], F32, tag=f"m{mh}")
                nc.vector.tensor_mul(m, Sg[:, :, 1, :], cs)              # f*c
                t = small.tile([P, 2, Bc], F32, tag=f"t{mh}")
                nc.vector.scalar_tensor_tensor(                          # (sg_g-.5)*i
                    out=t, in0=Sg[:, :, 3, :], scalar=-0.5, in1=Sg[:, :, 0, :],
                    op0=ALU.add, op1=ALU.mult)
                nc.vector.scalar_tensor_tensor(                          # c = 2t + m
                    out=cs, in0=t, scalar=2.0, in1=m,
                    op0=ALU.mult, op1=ALU.add)
                Sc = small.tile([P, 2, Bc], F32, tag=f"sc{mh}")
                nc.scalar.activation(Sc, cs, SIG, scale=2.0)             # sigmoid(2c)
                nc.vector.scalar_tensor_tensor(                          # h2=(Sc-.5)*o
                    out=h_new, in0=Sc, scalar=-0.5, in1=Sg[:, :, 2, :],
                    op0=ALU.add, op1=ALU.mult)

            hb_prev = [None]
            xs_t, o1_t, xgc_t = {}, {}, {}

            def load_inputs(kk):
                if kk >= iters:
                    return
                xs_t[kk] = xsp.tile([P, CHUNK * Bc], F16, tag="xs", name=f"xs{kk}")
                nc.sync.dma_start(
                    out=xs_t[kk],
                    in_=xm_in[:, kk * CHUNK * Bc:(kk + 1) * CHUNK * Bc])
                if kk >= LAG:
                    o1_t[kk] = o1p.tile([P, CHUNK, F], F16, tag="o1", name=f"o1_{kk}")
                    nc.scalar.dma_start(
                        out=o1_t[kk],
                        in_=out1_full[kk - LAG][0:1].rearrange(
                            "o p sf -> p (o sf)").rearrange(
                            "p (s f) -> p s f", s=CHUNK))

            def prod_mi(kk, mi, dep_rhs):
                """Produce xg chunk kk for m-block mi (all CHUNK steps at once).

                xg = wib@x_kk + wob@out1_{kk-LAG} + bias; exactly one of
                wib/wob is nonzero per core.  bias0 ensures phantom chunks
                (kk < LAG on L2 cores) keep h,c exactly zero.

                The first matmul adds an exact zero (zero1.T @ dep_rhs) but
                reads the current step's h tile: it pins the sweep to this
                step so the static scheduler cannot hoist all production to
                the chunk boundary (which left the recurrence bursts without
                PE filler work -> HAM-throttled cold matmuls mid-chunk).
                """
                if kk >= iters:
                    return
                xsv = xs_t[kk].rearrange("p (s b) -> p s b", s=CHUNK)
                pp = ppp.tile([P, CHUNK * Bc], F32, tag="pp")
                # ordering-only matmul: writes zeros to pp[:, :F]; the wib
                # matmul below (start=True over the full tile) overwrites
                # them, so the value contribution is nil -- only the FIFO
                # position matters.
                nc.tensor.matmul(
                    pp[:, 0:F], zero1, dep_rhs,
                    start=True, stop=True, skip_group_check=True)
                nc.tensor.matmul(
                    pp, wib[:, mi * P:(mi + 1) * P], xsv,
                    start=True, stop=(kk < LAG), skip_group_check=True)
                if kk >= LAG:
                    for j in range(NJ):
                        nc.tensor.matmul(
                            pp,
                            wob[:, (j * NM + mi) * P:(j * NM + mi + 1) * P],
                            o1_t[kk][:, :, j * Bc:(j + 1) * Bc],
                            start=False, stop=(j == NJ - 1), skip_group_check=True)
                return pp

            def prod_bias(kk, mi, pp):
                """Bias add for a production sweep.  Emitted AFTER the step's
                chain so the 512-col DVE op queues behind the critical-path
                chain ops instead of head-of-line blocking them."""
                if kk >= iters:
                    return
                nc.vector.tensor_scalar_add(
                    xgc_t[kk][:, :, mi, :],
                    pp.rearrange("p (s b) -> p s b", s=CHUNK),
                    (bias0 if kk < LAG else bias)[:, mi:mi + 1])

            # ---- prologue: chunk 0 inputs + production ----
            load_inputs(0)
            xgc_t[0] = xgp.tile([P, CHUNK, NM, Bc], F16, tag="xg", name="xg0")
            for mi in range(NM):
                pp0 = prod_mi(0, mi, whh[0:1, 0:F])
                prod_bias(0, mi, pp0)

            # ---- pipeline over iterations ----
            for k in range(iters):
                load_inputs(k + 1)
                if k + 1 < iters:
                    xgc_t[k + 1] = xgp.tile([P, CHUNK, NM, Bc], F16, tag="xg",
                                            name=f"xg{k+1}")
                xgc = xgc_t[k]
                for u in range(CHUNK // SUB_H):
                    hb = hrol.tile([P, SUB_H, NJ, Bc], F16, tag="h")
                    for s in range(SUB_H):
                        st = u * SUB_H + s
                        if hb_prev[0] is None and st == 0:
                            hp = zero_h[:, :].rearrange("p (j b) -> p j b", j=NJ)
                        elif s == 0:
                            hp = hb_prev[0][:, SUB_H - 1, :, :]
                        else:
                            hp = hb[:, s - 1, :, :]
                        Gt = step_mms(xgc[:, st, :, :], hp[:, 0:2, :], hp[:, 2:4, :])
                        # one production m-block per step fills the PE gap
                        # while this step's chain runs; gating it on the
                        # PREVIOUS h makes it ready exactly when this step's
                        # recurrence is, so it queues right behind it and
                        # cannot be hoisted to the chunk boundary.
                        ppx = prod_mi(k + 1, st,
                                      hp[0:1, :, :].rearrange("p j b -> p (j b)"))
                        chain_half(Gt, 0, hb[:, s, 0:2, :])
                        chain_half(Gt, 1, hb[:, s, 2:4, :])
                        prod_bias(k + 1, st, ppx)
                    hb_prev[0] = hb
                    nc.sync.dma_start(
                        out=out1_stage[k][:, u * SUB_H * F:(u + 1) * SUB_H * F].rearrange(
                            "p (s f) -> p s f", s=SUB_H),
                        in_=hb.rearrange("p s j b -> p s (j b)"))
                nc.gpsimd.collective_compute(
                    "AllGather", mybir.AluOpType.bypass,
                    replica_groups=PAIRS,
                    ins=[out1_stage[k][:, :]], outs=[out1_full[k][:, :, :]])
                xs_t.pop(k, None)
                o1_t.pop(k, None)
                xgc_t.pop(k, None)

            # ---- final outputs (h2 scaled to h on the host) ----
            h32 = state.tile([P, F], F32)
            nc.vector.tensor_copy(
                h32, hb_prev[0][:, SUB_H - 1, :, :].rearrange("p j b -> p (j b)"))
            nc.sync.dma_start(out=h_out[:, :], in_=h32)
            nc.sync.dma_start(
                out=c_out[:, :], in_=cT.rearrange("p j b -> p (j b)"))
    return nc


# ---------------- host-side packing ----------------

def _perm_rows():
    gate_base = {"i": 0, "f": H, "g": 2 * H, "o": 3 * H}
    order = []
    for j in range(NJ):
        for gname in ("i", "f", "o", "g"):
            start = gate_base[gname] + j * P
            order.extend(range(start, start + P))
    return np.array(order)


def _g_row_mask():
    """Mask (in permuted row order) of the g-gate rows (gidx==3)."""
    m = np.zeros(G4, bool)
    m[(np.arange(G4) // P) % 4 == 3] = True
    return m


def _pack_whh(W):
    Wr = W.reshape(NM, P, NJ, P)       # [mi, q, j, p]
    out = Wr.transpose(3, 2, 0, 1)     # [p, j, mi, q]
    return np.ascontiguousarray(out.reshape(P, NJ * NM * P)).astype(np.float16)


def pack_inputs(x, W_ih1, W_hh1, b_ih1, b_hh1, W_ih2, W_hh2, b_ih2, b_hh2, nch):
    iters = nch + LAG
    perm = _perm_rows()
    gmask = _g_row_mask()
    W_ih1 = np.asarray(W_ih1, np.float64)[perm]
    W_hh1 = np.asarray(W_hh1, np.float64)[perm]
    W_ih2 = np.asarray(W_ih2, np.float64)[perm]
    W_hh2 = np.asarray(W_hh2, np.float64)[perm]
    bias1 = (np.asarray(b_ih1, np.float64) + np.asarray(b_hh1, np.float64))[perm]
    bias2 = (np.asarray(b_ih2, np.float64) + np.asarray(b_hh2, np.float64))[perm]

    # tanh->sigmoid trick: double the g-gate rows of every pre-activation
    # contribution.
    for a in (W_ih1, W_hh1, W_ih2, W_hh2, bias1, bias2):
        a[gmask] *= 2.0
    # h2 = h/2 compensation: double the columns that consume h
    # (W_hh1/W_hh2 all columns, W_ih2 all columns; x is not h -> W_ih1 no).
    W_hh1 *= 2.0
    W_hh2 *= 2.0
    W_ih2 *= 2.0

    bias1 = bias1.astype(np.float32)
    bias2 = bias2.astype(np.float32)
    whh1 = _pack_whh(W_hh1)
    whh2 = _pack_whh(W_hh2)
    wob = _pack_whh(W_ih2)
    wib = np.ascontiguousarray(
        W_ih1.reshape(NM, P, D).transpose(2, 0, 1).reshape(D, NM * P)).astype(np.float16)

    b1p = np.ascontiguousarray(bias1.reshape(NM, P).T)
    b2p = np.ascontiguousarray(bias2.reshape(NM, P).T)
    bias0_l2 = np.full((P, NM), -30.0, np.float32)
    bias0_l2[:, 3::4] = -60.0  # g rows doubled
    ident = np.eye(P, dtype=np.float16)
    zeros_like = lambda a: np.zeros_like(a)

    x16 = np.asarray(x).astype(np.float16)
    T_steps = nch * CHUNK
    in_maps = []
    for c in range(N_CORES):
        q = c % 4
        xq = x16[q * Bc:(q + 1) * Bc, :T_steps, :]          # [Bc, T, D]
        xm = np.zeros((D, iters * CHUNK * Bc), np.float16)
        if c < 4:
            xm[:, :T_steps * Bc] = xq.transpose(2, 1, 0).reshape(D, T_steps * Bc)
        is_l1 = c < 4
        in_maps.append({
            "x_my": xm,
            "whh": whh1 if is_l1 else whh2,
            "wib": wib if is_l1 else zeros_like(wib),
            "wob": zeros_like(wob) if is_l1 else wob,
            "bias": b1p if is_l1 else b2p,
            "bias0": b1p if is_l1 else bias0_l2,
            "ident": ident,
            "roles": np.array([[1 if is_l1 else 0, 0 if is_l1 else 1, q]], np.uint32),
        })
    return in_maps


def unpack_outputs(results):
    hs, cs = [], []
    for q in range(4):
        for name, acc, scale in (("h_out", hs, 2.0), ("c_out", cs, 1.0)):
            a = results[4 + q][name] * scale
            acc.append(np.ascontiguousarray(
                a.reshape(P, NJ, Bc).transpose(2, 1, 0).reshape(Bc, H)))
    return np.concatenate(hs)[None], np.concatenate(cs)[None]


# ---------------- harness entry point ----------------

N_CHUNKS_FULL = NCH_FULL = 32  # T = 512 (nch chunks of CHUNK=16)

_CACHE = {}


def _get_nc():
    if "nc" not in _CACHE:
        nc = build(NCH_FULL)
        nc.finalize()
        _CACHE["nc"] = nc
    return _CACHE["nc"]


def kernel(x, W_ih1, W_hh1, b_ih1, b_hh1, W_ih2, W_hh2, b_ih2, b_hh2):
    """Full (unsharded) inputs -> (h_T [1, B, H], c_T [1, B, H]) fp32."""
    import time as _time
    from concourse.bass_utils import run_bass_kernel_spmd
    nc = _get_nc()
    in_maps = pack_inputs(x, W_ih1, W_hh1, b_ih1, b_hh1,
                          W_ih2, W_hh2, b_ih2, b_hh2, NCH_FULL)
    last_err = None
    for attempt in range(3):
        try:
            res = run_bass_kernel_spmd(nc, in_maps, CORE_IDS)
            h, c = unpack_outputs(res.results)
            return h.astype(np.float32), c.astype(np.float32)
        except Exception as e:  # transient device wedge: back off and retry
            last_err = e
            _time.sleep(5 * (attempt + 1))
    raise last_err


if __name__ == "__main__":
    # CoreSim verification at small T
    nch = int(sys.argv[1]) if len(sys.argv) > 1 else 2
    T_steps = nch * CHUNK
    rng = np.random.default_rng(0)
    s1 = 1.0 / np.sqrt(H)
    x = rng.standard_normal((B, T_steps, D), dtype=np.float32)
    W_ih1 = rng.uniform(-s1, s1, (G4, D)).astype(np.float32)
    W_hh1 = rng.uniform(-s1, s1, (G4, H)).astype(np.float32)
    b_ih1 = rng.uniform(-s1, s1, G4).astype(np.float32)
    b_hh1 = rng.uniform(-s1, s1, G4).astype(np.float32)
    W_ih2 = rng.uniform(-s1, s1, (G4, H)).astype(np.float32)
    W_hh2 = rng.uniform(-s1, s1, (G4, H)).astype(np.float32)
    b_ih2 = rng.uniform(-s1, s1, G4).astype(np.float32)
    b_hh2 = rng.uniform(-s1, s1, G4).astype(np.float32)

    def np_lstm(x, W_ih, W_hh, b):
        Bs, T, _ = x.shape
        Hn = W_hh.shape[1]
        h = np.zeros((Bs, Hn), np.float32)
        c = np.zeros((Bs, Hn), np.float32)
        outs = np.zeros((Bs, T, Hn), np.float32)
        xg = x @ W_ih.T + b
        sig = lambda v: 1.0 / (1.0 + np.exp(-v))
        for t in range(T):
            g = xg[:, t] + h @ W_hh.T
            i, f, gg, o = np.split(g, 4, axis=-1)
            i, f, o = sig(i), sig(f), sig(o)
            gg = np.tanh(gg)
            c = f * c + i * gg
            h = o * np.tanh(c)
            outs[:, t] = h
        return outs, h, c

    o1, _, _ = np_lstm(x, W_ih1, W_hh1, b_ih1 + b_hh1)
    _, h_ref, c_ref = np_lstm(o1, W_ih2, W_hh2, b_ih2 + b_hh2)

    nc = build(nch)
    nc.finalize()
    in_maps = pack_inputs(x, W_ih1, W_hh1, b_ih1, b_hh1,
                          W_ih2, W_hh2, b_ih2, b_hh2, nch)

    from concourse.bass_interp import MultiCoreSim
    sim = MultiCoreSim(nc, num_cores=N_CORES, require_finite=False,
                       require_nnan=False)
    for cid, core_sim in sim.cores.items():
        for name, val in in_maps[cid].items():
            core_sim.tensor(name)[:] = val
    sim.simulate()
    results = [{n: np.asarray(sim.cores[c].tensor(n)) for n in ("h_out", "c_out")}
               for c in range(N_CORES)]
    h_got, c_got = unpack_outputs(results)
    for name, got, exp in (("h", h_got[0], h_ref), ("c", c_got[0], c_ref)):
        err = np.abs(got - exp).max()
        scale = np.abs(exp).max()
        print(f"{name}: absmax={err:.3e} scale={scale:.3f} rel={err/scale:.3e}")


# revision 22
# speedup vs baseline: 1.0983x; 1.0983x over previous
"""Bass kernel v3 for 2-layer LSTM encoder (B=128, T=512, D=128, H=512).

Architecture (8 cores, batch-quarter x layer split), same as v2:
  cores 0-3: layer-1 recurrence for batch quarter q=cid   (Bc=32)
  cores 4-7: layer-2 recurrence for batch quarter q=cid-4 (lag LAG chunks)
Pairs (q, q+4) exchange layer-1 output chunks via AllGather.  All cores
execute ONE uniform instruction stream (role differences are data only):
xg = wib@x + wob@out1 + bias with exactly one of wib/wob nonzero per
core.

v3 changes (the v2 profile showed the step period was bound by the
serial ACT/DVE chain, ~4.0us/step, not the matmul stream ~0.7us):
  * tanh eliminated via tanh(x) = 2*sigmoid(2x) - 1.  The g-gate rows of
    all weight/bias tensors are pre-scaled by 2 on the host, so ONE
    sigmoid instruction covers all four gates of a half-step.  The cell
    update becomes  c = f*c + 2*(sg_g - 0.5)*sg_i  (2 fused DVE ops +
    1 mul), and the hidden state is kept as  h2 = h/2 =
    (sigmoid(2c) - 0.5)*sg_o  (1 fused DVE op).  The missing 2x on h is
    folded into the CONSUMERS of h: whh columns and wob (=W_ih2)
    columns are pre-scaled by 2, and the final h output is scaled by 2
    on the host.  ACT: 6 instrs/step -> 4; DVE: ~8 plain -> 8 fused-or-
    cheap ops/step.
  * xg production runs one full chunk per sweep (SUB=16): one
    [128,512]-col matmul per (mi, source) instead of four, one bias add
    per mi per chunk.
  * step matmul order: inject xg first (off critical path), then j=0,1
    for all mi (needs only h half lo), then j=2,3 for mi 0..7 (half-lo
    gates complete -> chain starts), then j=2,3 for mi 8..15.

On-chip layout is "transposed": tiles are [128 partitions, free].
  h2   -> rolling SBUF tiles [P, (s, j, b)] fp16, SUB_H=4 steps per tile
  c    -> SBUF [P, (j, b)] fp32
  gates -> PSUM [P, (mi, b)] fp32, one bank per step
m-chunk order (host permutes weight rows): mi = 4*j + {i,f,o,g}.
"half lo" = j in {0,1} = mi in [0,8); "hi" = j in {2,3} = mi in [8,16).
"""
import sys
sys.path.insert(0, "/opt/trn_rl_repo")
import numpy as np
from concourse import bacc
import concourse.bass as bass
import concourse.mybir as mybir
import concourse.tile as tile

F16 = mybir.dt.float16
F32 = mybir.dt.float32
U32 = mybir.dt.uint32

N_CORES = 8
CORE_IDS = list(range(N_CORES))
P = 128
B = 128
D = 128
H = 512
G4 = 2048
NJ = H // P          # 4
NM = G4 // P         # 16
Bc = B // 4          # 32 batch per core
F = NJ * Bc          # 128 state cols per core
CHUNK = 16           # steps per chunk
SUB_H = 4            # steps per rolling h tile (DMA granularity)

SIG = mybir.ActivationFunctionType.Sigmoid
ALU = mybir.AluOpType

LAG = 3  # L2 processes chunk k-LAG at iteration k


def build(nch):
    """nch = number of real chunks (T = nch*CHUNK); ITERS = nch+LAG."""
    iters = nch + LAG
    nc = bacc.Bacc()

    # ---------------- inputs ----------------
    xm_in = nc.declare_dram_parameter("x_my", [D, iters * CHUNK * Bc], F16, isOutput=False)
    whh_in = nc.declare_dram_parameter("whh", [P, NJ * NM * P], F16, isOutput=False)
    wib_in = nc.declare_dram_parameter("wib", [P, NM * P], F16, isOutput=False)
    wob_in = nc.declare_dram_parameter("wob", [P, NJ * NM * P], F16, isOutput=False)
    bias_in = nc.declare_dram_parameter("bias", [P, NM], F32, isOutput=False)
    bias0_in = nc.declare_dram_parameter("bias0", [P, NM], F32, isOutput=False)
    ident_in = nc.declare_dram_parameter("ident", [P, P], F16, isOutput=False)
    roles_in = nc.declare_dram_parameter("roles", [1, 3], U32, isOutput=False)

    # ---------------- outputs ----------------
    h_out = nc.declare_dram_parameter("h_out", [P, F], F32, isOutput=True)
    c_out = nc.declare_dram_parameter("c_out", [P, F], F32, isOutput=True)

    # ---------------- internal DRAM ----------------
    out1_stage = [nc.dram_tensor(f"o1s{k}", [P, CHUNK * F], F16) for k in range(iters)]
    out1_full = [nc.dram_tensor(f"o1f{k}", [N_CORES, P, CHUNK * F], F16,
                                addr_space="Shared")
                 for k in range(iters)]

    with tile.TileContext(nc) as tc:
        with (
            tc.tile_pool(name="wpool", bufs=1) as wpool,
            tc.tile_pool(name="xgp", bufs=2) as xgp,
            tc.tile_pool(name="xsp", bufs=2) as xsp,
            tc.tile_pool(name="o1p", bufs=2) as o1p,
            tc.tile_pool(name="hrol", bufs=2) as hrol,
            tc.tile_pool(name="state", bufs=1) as state,
            tc.tile_pool(name="actp", bufs=2) as actp,
            tc.tile_pool(name="small", bufs=2) as small,
            tc.tile_pool(name="ps", bufs=2, space="PSUM") as psp,
            tc.tile_pool(name="pp", bufs=2, space="PSUM") as ppp,
        ):
            # ---- constants ----
            whh = wpool.tile([P, NJ * NM * P], F16)
            wib = wpool.tile([P, NM * P], F16)
            wob = wpool.tile([P, NJ * NM * P], F16)
            bias = wpool.tile([P, NM], F32)
            bias0 = wpool.tile([P, NM], F32)
            ident = wpool.tile([P, P], F16)
            zero1 = wpool.tile([1, P], F16)
            roles_t = wpool.tile([1, 3], U32)
            nc.vector.memset(zero1, 0.0)
            nc.sync.dma_start(out=whh, in_=whh_in[:, :])
            nc.sync.dma_start(out=wib, in_=wib_in[:, :])
            nc.sync.dma_start(out=wob, in_=wob_in[:, :])
            nc.sync.dma_start(out=bias, in_=bias_in[:, :])
            nc.sync.dma_start(out=bias0, in_=bias0_in[:, :])
            nc.sync.dma_start(out=ident, in_=ident_in[:, :])
            nc.sync.dma_start(out=roles_t, in_=roles_in[:, :])
            slot_reg = nc.scalar.alloc_register("slot_reg")
            nc.scalar.reg_load(slot_reg, roles_t[0:1, 2:3])
            slot = nc.scalar.snap(slot_reg, min_val=0, max_val=3)

            # ---- state ----
            cT = state.tile([P, NJ, Bc], F32)
            zero_h = state.tile([P, F], F16)
            nc.vector.memset(cT, 0.0)
            nc.vector.memset(zero_h, 0.0)

            # ---- one recurrence step ----
            # G layout: [P, mi(16), Bc]; mi = 4*j + {i,f,o,g}
            def step_mms(xg_s, h_lo, h_hi):
                """Emit the step's matmuls. h_lo/h_hi: [P, 2, Bc] fp16 views
                (j=0,1 and j=2,3 of the previous step's h2)."""
                Gt = psp.tile([P, NM * Bc], F32, tag="gates")
                nc.tensor.matmul(Gt, ident, xg_s, start=True, stop=False,
                                 skip_group_check=True)
                for j in range(2):
                    for mi in range(NM):
                        nc.tensor.matmul(
                            Gt[:, mi * Bc:(mi + 1) * Bc],
                            whh[:, (j * NM + mi) * P:(j * NM + mi + 1) * P],
                            h_lo[:, j, :],
                            start=False, stop=False, skip_group_check=True)
                for mh in range(2):
                    for mi in range(mh * 8, mh * 8 + 8):
                        for j in range(2, 4):
                            nc.tensor.matmul(
                                Gt[:, mi * Bc:(mi + 1) * Bc],
                                whh[:, (j * NM + mi) * P:(j * NM + mi + 1) * P],
                                h_hi[:, j - 2, :],
                                start=False, stop=(j == 3), skip_group_check=True)
                return Gt

            def chain_half(Gt, mh, h_new):
                """Elementwise chain for half mh (j in {2mh, 2mh+1}).
                h_new: [P, 2, Bc] fp16 view to write h2 = h/2 into."""
                Gh = Gt[:, mh * 8 * Bc:(mh + 1) * 8 * Bc].rearrange(
                    "p (j g b) -> p j g b", j=2, g=4)
                Sg = actp.tile([P, 2, 4, Bc], F32, tag=f"S{mh}")
                nc.scalar.activation(Sg, Gh, SIG)
                cs = cT[:, 2 * mh:2 * mh + 2, :]
                m = small.tile([P, 2, Bc], F32, tag=f"m{mh}")
                nc.vector.tensor_mul(m, Sg[:, :, 1, :], cs)              # f*c
                t = small.tile([P, 2, Bc], F32, tag=f"t{mh}")
                nc.vector.scalar_tensor_tensor(                          # (sg_g-.5)*i
                    out=t, in0=Sg[:, :, 3, :], scalar=-0.5, in1=Sg[:, :, 0, :],
                    op0=ALU.add, op1=ALU.mult)
                nc.vector.scalar_tensor_tensor(                          # c = 2t + m
                    out=cs, in0=t, scalar=2.0, in1=m,
                    op0=ALU.mult, op1=ALU.add)
                Sc = small.tile([P, 2, Bc], F32, tag=f"sc{mh}")
                nc.scalar.activation(Sc, cs, SIG, scale=2.0)             # sigmoid(2c)
                nc.vector.scalar_tensor_tensor(                          # h2=(Sc-.5)*o
                    out=h_new, in0=Sc, scalar=-0.5, in1=Sg[:, :, 2, :],
                    op0=ALU.add, op1=ALU.mult)

            hb_prev = [None]
            xs_t, o1_t, xgc_t = {}, {}, {}

            def load_inputs(kk):
                if kk >= iters:
                    return
                xs_t[kk] = xsp.tile([P, CHUNK * Bc], F16, tag="xs", name=f"xs{kk}")
                nc.sync.dma_start(
                    out=xs_t[kk],
                    in_=xm_in[:, kk * CHUNK * Bc:(kk + 1) * CHUNK * Bc])
                if kk >= LAG:
                    o1_t[kk] = o1p.tile([P, CHUNK, F], F16, tag="o1", name=f"o1_{kk}")
                    nc.scalar.dma_start(
                        out=o1_t[kk],
                        in_=out1_full[kk - LAG][bass.ds(slot, 1)].rearrange(
                            "o p sf -> p (o sf)").rearrange(
                            "p (s f) -> p s f", s=CHUNK))

            def prod_mi(kk, mi, dep_rhs):
                """Produce xg chunk kk for m-block mi (all CHUNK steps at once).

                xg = wib@x_kk + wob@out1_{kk-LAG} + bias; exactly one of
                wib/wob is nonzero per core.  bias0 ensures phantom chunks
                (kk < LAG on L2 cores) keep h,c exactly zero.

                The first matmul adds an exact zero (zero1.T @ dep_rhs) but
                reads the current step's h tile: it pins the sweep to this
                step so the static scheduler cannot hoist all production to
                the chunk boundary (which left the recurrence bursts without
                PE filler work -> HAM-throttled cold matmuls mid-chunk).
                """
                if kk >= iters:
                    return
                xsv = xs_t[kk].rearrange("p (s b) -> p s b", s=CHUNK)
                pp = ppp.tile([P, CHUNK * Bc], F32, tag="pp")
                # ordering-only matmul: writes zeros to pp[:, :F]; the wib
                # matmul below (start=True over the full tile) overwrites
                # them, so the value contribution is nil -- only the FIFO
                # position matters.
                nc.tensor.matmul(
                    pp[:, 0:F], zero1, dep_rhs,
                    start=True, stop=True, skip_group_check=True)
                nc.tensor.matmul(
                    pp, wib[:, mi * P:(mi + 1) * P], xsv,
                    start=True, stop=(kk < LAG), skip_group_check=True)
                if kk >= LAG:
                    for j in range(NJ):
                        nc.tensor.matmul(
                            pp,
                            wob[:, (j * NM + mi) * P:(j * NM + mi + 1) * P],
                            o1_t[kk][:, :, j * Bc:(j + 1) * Bc],
                            start=False, stop=(j == NJ - 1), skip_group_check=True)
                return pp

            def prod_bias(kk, mi, pp):
                """Bias add for a production sweep.  Emitted AFTER the step's
                chain so the 512-col DVE op queues behind the critical-path
                chain ops instead of head-of-line blocking them."""
                if kk >= iters:
                    return
                nc.vector.tensor_scalar_add(
                    xgc_t[kk][:, :, mi, :],
                    pp.rearrange("p (s b) -> p s b", s=CHUNK),
                    (bias0 if kk < LAG else bias)[:, mi:mi + 1])

            # ---- prologue: chunk 0 inputs + production ----
            load_inputs(0)
            xgc_t[0] = xgp.tile([P, CHUNK, NM, Bc], F16, tag="xg", name="xg0")
            for mi in range(NM):
                pp0 = prod_mi(0, mi, whh[0:1, 0:F])
                prod_bias(0, mi, pp0)

            # ---- pipeline over iterations ----
            for k in range(iters):
                load_inputs(k + 1)
                if k + 1 < iters:
                    xgc_t[k + 1] = xgp.tile([P, CHUNK, NM, Bc], F16, tag="xg",
                                            name=f"xg{k+1}")
                xgc = xgc_t[k]
                for u in range(CHUNK // SUB_H):
                    hb = hrol.tile([P, SUB_H, NJ, Bc], F16, tag="h")
                    for s in range(SUB_H):
                        st = u * SUB_H + s
                        if hb_prev[0] is None and st == 0:
                            hp = zero_h[:, :].rearrange("p (j b) -> p j b", j=NJ)
                        elif s == 0:
                            hp = hb_prev[0][:, SUB_H - 1, :, :]
                        else:
                            hp = hb[:, s - 1, :, :]
                        Gt = step_mms(xgc[:, st, :, :], hp[:, 0:2, :], hp[:, 2:4, :])
                        # one production m-block per step fills the PE gap
                        # while this step's chain runs; gating it on the
                        # PREVIOUS h makes it ready exactly when this step's
                        # recurrence is, so it queues right behind it and
                        # cannot be hoisted to the chunk boundary.
                        ppx = prod_mi(k + 1, st,
                                      hp[0:1, :, :].rearrange("p j b -> p (j b)"))
                        chain_half(Gt, 0, hb[:, s, 0:2, :])
                        chain_half(Gt, 1, hb[:, s, 2:4, :])
                        prod_bias(k + 1, st, ppx)
                    hb_prev[0] = hb
                    nc.sync.dma_start(
                        out=out1_stage[k][:, u * SUB_H * F:(u + 1) * SUB_H * F].rearrange(
                            "p (s f) -> p s f", s=SUB_H),
                        in_=hb.rearrange("p s j b -> p s (j b)"))
                nc.gpsimd.collective_compute(
                    "AllGather", mybir.AluOpType.bypass,
                    replica_groups=[CORE_IDS],
                    ins=[out1_stage[k][:, :]], outs=[out1_full[k][:, :, :]])
                xs_t.pop(k, None)
                o1_t.pop(k, None)
                xgc_t.pop(k, None)

            # ---- final outputs (h2 scaled to h on the host) ----
            h32 = state.tile([P, F], F32)
            nc.vector.tensor_copy(
                h32, hb_prev[0][:, SUB_H - 1, :, :].rearrange("p j b -> p (j b)"))
            nc.sync.dma_start(out=h_out[:, :], in_=h32)
            nc.sync.dma_start(
                out=c_out[:, :], in_=cT.rearrange("p j b -> p (j b)"))
    return nc


# ---------------- host-side packing ----------------

def _perm_rows():
    gate_base = {"i": 0, "f": H, "g": 2 * H, "o": 3 * H}
    order = []
    for j in range(NJ):
        for gname in ("i", "f", "o", "g"):
            start = gate_base[gname] + j * P
            order.extend(range(start, start + P))
    return np.array(order)


def _g_row_mask():
    """Mask (in permuted row order) of the g-gate rows (gidx==3)."""
    m = np.zeros(G4, bool)
    m[(np.arange(G4) // P) % 4 == 3] = True
    return m


def _pack_whh(W):
    Wr = W.reshape(NM, P, NJ, P)       # [mi, q, j, p]
    out = Wr.transpose(3, 2, 0, 1)     # [p, j, mi, q]
    return np.ascontiguousarray(out.reshape(P, NJ * NM * P)).astype(np.float16)


def pack_inputs(x, W_ih1, W_hh1, b_ih1, b_hh1, W_ih2, W_hh2, b_ih2, b_hh2, nch):
    iters = nch + LAG
    perm = _perm_rows()
    gmask = _g_row_mask()
    W_ih1 = np.asarray(W_ih1, np.float64)[perm]
    W_hh1 = np.asarray(W_hh1, np.float64)[perm]
    W_ih2 = np.asarray(W_ih2, np.float64)[perm]
    W_hh2 = np.asarray(W_hh2, np.float64)[perm]
    bias1 = (np.asarray(b_ih1, np.float64) + np.asarray(b_hh1, np.float64))[perm]
    bias2 = (np.asarray(b_ih2, np.float64) + np.asarray(b_hh2, np.float64))[perm]

    # tanh->sigmoid trick: double the g-gate rows of every pre-activation
    # contribution.
    for a in (W_ih1, W_hh1, W_ih2, W_hh2, bias1, bias2):
        a[gmask] *= 2.0
    # h2 = h/2 compensation: double the columns that consume h
    # (W_hh1/W_hh2 all columns, W_ih2 all columns; x is not h -> W_ih1 no).
    W_hh1 *= 2.0
    W_hh2 *= 2.0
    W_ih2 *= 2.0

    bias1 = bias1.astype(np.float32)
    bias2 = bias2.astype(np.float32)
    whh1 = _pack_whh(W_hh1)
    whh2 = _pack_whh(W_hh2)
    wob = _pack_whh(W_ih2)
    wib = np.ascontiguousarray(
        W_ih1.reshape(NM, P, D).transpose(2, 0, 1).reshape(D, NM * P)).astype(np.float16)

    b1p = np.ascontiguousarray(bias1.reshape(NM, P).T)
    b2p = np.ascontiguousarray(bias2.reshape(NM, P).T)
    bias0_l2 = np.full((P, NM), -30.0, np.float32)
    bias0_l2[:, 3::4] = -60.0  # g rows doubled
    ident = np.eye(P, dtype=np.float16)
    zeros_like = lambda a: np.zeros_like(a)

    x16 = np.asarray(x).astype(np.float16)
    T_steps = nch * CHUNK
    in_maps = []
    for c in range(N_CORES):
        q = c % 4
        xq = x16[q * Bc:(q + 1) * Bc, :T_steps, :]          # [Bc, T, D]
        xm = np.zeros((D, iters * CHUNK * Bc), np.float16)
        if c < 4:
            xm[:, :T_steps * Bc] = xq.transpose(2, 1, 0).reshape(D, T_steps * Bc)
        is_l1 = c < 4
        in_maps.append({
            "x_my": xm,
            "whh": whh1 if is_l1 else whh2,
            "wib": wib if is_l1 else zeros_like(wib),
            "wob": zeros_like(wob) if is_l1 else wob,
            "bias": b1p if is_l1 else b2p,
            "bias0": b1p if is_l1 else bias0_l2,
            "ident": ident,
            "roles": np.array([[1 if is_l1 else 0, 0 if is_l1 else 1, q]], np.uint32),
        })
    return in_maps


def unpack_outputs(results):
    hs, cs = [], []
    for q in range(4):
        for name, acc, scale in (("h_out", hs, 2.0), ("c_out", cs, 1.0)):
            a = results[4 + q][name] * scale
            acc.append(np.ascontiguousarray(
                a.reshape(P, NJ, Bc).transpose(2, 1, 0).reshape(Bc, H)))
    return np.concatenate(hs)[None], np.concatenate(cs)[None]


# ---------------- harness entry point ----------------

N_CHUNKS_FULL = NCH_FULL = 32  # T = 512 (nch chunks of CHUNK=16)

_CACHE = {}


def _get_nc():
    if "nc" not in _CACHE:
        nc = build(NCH_FULL)
        nc.finalize()
        _CACHE["nc"] = nc
    return _CACHE["nc"]


def kernel(x, W_ih1, W_hh1, b_ih1, b_hh1, W_ih2, W_hh2, b_ih2, b_hh2):
    """Full (unsharded) inputs -> (h_T [1, B, H], c_T [1, B, H]) fp32."""
    import time as _time
    from concourse.bass_utils import run_bass_kernel_spmd
    nc = _get_nc()
    in_maps = pack_inputs(x, W_ih1, W_hh1, b_ih1, b_hh1,
                          W_ih2, W_hh2, b_ih2, b_hh2, NCH_FULL)
    last_err = None
    for attempt in range(3):
        try:
            res = run_bass_kernel_spmd(nc, in_maps, CORE_IDS)
            h, c = unpack_outputs(res.results)
            return h.astype(np.float32), c.astype(np.float32)
        except Exception as e:  # transient device wedge: back off and retry
            last_err = e
            _time.sleep(5 * (attempt + 1))
    raise last_err


if __name__ == "__main__":
    # CoreSim verification at small T
    nch = int(sys.argv[1]) if len(sys.argv) > 1 else 2
    T_steps = nch * CHUNK
    rng = np.random.default_rng(0)
    s1 = 1.0 / np.sqrt(H)
    x = rng.standard_normal((B, T_steps, D), dtype=np.float32)
    W_ih1 = rng.uniform(-s1, s1, (G4, D)).astype(np.float32)
    W_hh1 = rng.uniform(-s1, s1, (G4, H)).astype(np.float32)
    b_ih1 = rng.uniform(-s1, s1, G4).astype(np.float32)
    b_hh1 = rng.uniform(-s1, s1, G4).astype(np.float32)
    W_ih2 = rng.uniform(-s1, s1, (G4, H)).astype(np.float32)
    W_hh2 = rng.uniform(-s1, s1, (G4, H)).astype(np.float32)
    b_ih2 = rng.uniform(-s1, s1, G4).astype(np.float32)
    b_hh2 = rng.uniform(-s1, s1, G4).astype(np.float32)

    def np_lstm(x, W_ih, W_hh, b):
        Bs, T, _ = x.shape
        Hn = W_hh.shape[1]
        h = np.zeros((Bs, Hn), np.float32)
        c = np.zeros((Bs, Hn), np.float32)
        outs = np.zeros((Bs, T, Hn), np.float32)
        xg = x @ W_ih.T + b
        sig = lambda v: 1.0 / (1.0 + np.exp(-v))
        for t in range(T):
            g = xg[:, t] + h @ W_hh.T
            i, f, gg, o = np.split(g, 4, axis=-1)
            i, f, o = sig(i), sig(f), sig(o)
            gg = np.tanh(gg)
            c = f * c + i * gg
            h = o * np.tanh(c)
            outs[:, t] = h
        return outs, h, c

    o1, _, _ = np_lstm(x, W_ih1, W_hh1, b_ih1 + b_hh1)
    _, h_ref, c_ref = np_lstm(o1, W_ih2, W_hh2, b_ih2 + b_hh2)

    nc = build(nch)
    nc.finalize()
    in_maps = pack_inputs(x, W_ih1, W_hh1, b_ih1, b_hh1,
                          W_ih2, W_hh2, b_ih2, b_hh2, nch)

    from concourse.bass_interp import MultiCoreSim
    sim = MultiCoreSim(nc, num_cores=N_CORES, require_finite=False,
                       require_nnan=False)
    for cid, core_sim in sim.cores.items():
        for name, val in in_maps[cid].items():
            core_sim.tensor(name)[:] = val
    sim.simulate()
    results = [{n: np.asarray(sim.cores[c].tensor(n)) for n in ("h_out", "c_out")}
               for c in range(N_CORES)]
    h_got, c_got = unpack_outputs(results)
    for name, got, exp in (("h", h_got[0], h_ref), ("c", c_got[0], c_ref)):
        err = np.abs(got - exp).max()
        scale = np.abs(exp).max()
        print(f"{name}: absmax={err:.3e} scale={scale:.3f} rel={err/scale:.3e}")


# revision 24
# speedup vs baseline: 1.1067x; 1.0076x over previous
"""Bass kernel v3 for 2-layer LSTM encoder (B=128, T=512, D=128, H=512).

Architecture (8 cores, batch-quarter x layer split), same as v2:
  cores 0-3: layer-1 recurrence for batch quarter q=cid   (Bc=32)
  cores 4-7: layer-2 recurrence for batch quarter q=cid-4 (lag LAG chunks)
Pairs (q, q+4) exchange layer-1 output chunks via AllGather.  All cores
execute ONE uniform instruction stream (role differences are data only):
xg = wib@x + wob@out1 + bias with exactly one of wib/wob nonzero per
core.

v3 changes (the v2 profile showed the step period was bound by the
serial ACT/DVE chain, ~4.0us/step, not the matmul stream ~0.7us):
  * tanh eliminated via tanh(x) = 2*sigmoid(2x) - 1.  The g-gate rows of
    all weight/bias tensors are pre-scaled by 2 on the host, so ONE
    sigmoid instruction covers all four gates of a half-step.  The cell
    update becomes  c = f*c + 2*(sg_g - 0.5)*sg_i  (2 fused DVE ops +
    1 mul), and the hidden state is kept as  h2 = h/2 =
    (sigmoid(2c) - 0.5)*sg_o  (1 fused DVE op).  The missing 2x on h is
    folded into the CONSUMERS of h: whh columns and wob (=W_ih2)
    columns are pre-scaled by 2, and the final h output is scaled by 2
    on the host.  ACT: 6 instrs/step -> 4; DVE: ~8 plain -> 8 fused-or-
    cheap ops/step.
  * xg production runs one full chunk per sweep (SUB=16): one
    [128,512]-col matmul per (mi, source) instead of four, one bias add
    per mi per chunk.
  * step matmul order: inject xg first (off critical path), then j=0,1
    for all mi (needs only h half lo), then j=2,3 for mi 0..7 (half-lo
    gates complete -> chain starts), then j=2,3 for mi 8..15.

On-chip layout is "transposed": tiles are [128 partitions, free].
  h2   -> rolling SBUF tiles [P, (s, j, b)] fp16, SUB_H=4 steps per tile
  c    -> SBUF [P, (j, b)] fp32
  gates -> PSUM [P, (mi, b)] fp32, one bank per step
m-chunk order (host permutes weight rows): mi = 4*j + {i,f,o,g}.
"half lo" = j in {0,1} = mi in [0,8); "hi" = j in {2,3} = mi in [8,16).
"""
import sys
sys.path.insert(0, "/opt/trn_rl_repo")
import numpy as np
from concourse import bacc
import concourse.bass as bass
import concourse.mybir as mybir
import concourse.tile as tile

F16 = mybir.dt.float16
F32 = mybir.dt.float32
U32 = mybir.dt.uint32

N_CORES = 8
CORE_IDS = list(range(N_CORES))
P = 128
B = 128
D = 128
H = 512
G4 = 2048
NJ = H // P          # 4
NM = G4 // P         # 16
Bc = B // 4          # 32 batch per core
F = NJ * Bc          # 128 state cols per core
CHUNK = 16           # steps per chunk
SUB_H = 4            # steps per rolling h tile (DMA granularity)

SIG = mybir.ActivationFunctionType.Sigmoid
ALU = mybir.AluOpType

LAG = 3  # L2 processes chunk k-LAG at iteration k


def build(nch):
    """nch = number of real chunks (T = nch*CHUNK); ITERS = nch+LAG."""
    iters = nch + LAG
    nc = bacc.Bacc()

    # ---------------- inputs ----------------
    xm_in = nc.declare_dram_parameter("x_my", [D, iters * CHUNK * Bc], F16, isOutput=False)
    whh_in = nc.declare_dram_parameter("whh", [P, NJ * NM * P], F16, isOutput=False)
    wib_in = nc.declare_dram_parameter("wib", [P, NM * P], F16, isOutput=False)
    wob_in = nc.declare_dram_parameter("wob", [P, NJ * NM * P], F16, isOutput=False)
    bias_in = nc.declare_dram_parameter("bias", [P, NM], F32, isOutput=False)
    bias0_in = nc.declare_dram_parameter("bias0", [P, NM], F32, isOutput=False)
    ident_in = nc.declare_dram_parameter("ident", [P, P], F16, isOutput=False)
    roles_in = nc.declare_dram_parameter("roles", [1, 3], U32, isOutput=False)

    # ---------------- outputs ----------------
    h_out = nc.declare_dram_parameter("h_out", [P, F], F32, isOutput=True)
    c_out = nc.declare_dram_parameter("c_out", [P, F], F32, isOutput=True)

    # ---------------- internal DRAM ----------------
    out1_stage = [nc.dram_tensor(f"o1s{k}", [P, CHUNK * F], F16) for k in range(iters)]
    out1_full = [nc.dram_tensor(f"o1f{k}", [N_CORES, P, CHUNK * F], F16,
                                addr_space="Shared")
                 for k in range(iters)]

    with tile.TileContext(nc) as tc:
        with (
            tc.tile_pool(name="wpool", bufs=1) as wpool,
            tc.tile_pool(name="xgp", bufs=2) as xgp,
            tc.tile_pool(name="xsp", bufs=2) as xsp,
            tc.tile_pool(name="o1p", bufs=2) as o1p,
            tc.tile_pool(name="hrol", bufs=2) as hrol,
            tc.tile_pool(name="state", bufs=1) as state,
            tc.tile_pool(name="actp", bufs=2) as actp,
            tc.tile_pool(name="small", bufs=2) as small,
            tc.tile_pool(name="ps", bufs=2, space="PSUM") as psp,
            tc.tile_pool(name="pp", bufs=2, space="PSUM") as ppp,
        ):
            # ---- constants ----
            whh = wpool.tile([P, NJ * NM * P], F16)
            wib = wpool.tile([P, NM * P], F16)
            wob = wpool.tile([P, NJ * NM * P], F16)
            bias = wpool.tile([P, NM], F32)
            bias0 = wpool.tile([P, NM], F32)
            ident = wpool.tile([P, P], F16)
            zero1 = wpool.tile([1, P], F16)
            roles_t = wpool.tile([1, 3], U32)
            nc.vector.memset(zero1, 0.0)
            nc.sync.dma_start(out=whh, in_=whh_in[:, :])
            nc.sync.dma_start(out=wib, in_=wib_in[:, :])
            nc.sync.dma_start(out=wob, in_=wob_in[:, :])
            nc.sync.dma_start(out=bias, in_=bias_in[:, :])
            nc.sync.dma_start(out=bias0, in_=bias0_in[:, :])
            nc.sync.dma_start(out=ident, in_=ident_in[:, :])
            nc.sync.dma_start(out=roles_t, in_=roles_in[:, :])
            slot_reg = nc.gpsimd.alloc_register("slot_reg")
            nc.gpsimd.reg_load(slot_reg, roles_t[0:1, 2:3])
            slot = nc.gpsimd.snap(slot_reg, min_val=0, max_val=3)

            # ---- state ----
            cT = state.tile([P, NJ, Bc], F32)
            zero_h = state.tile([P, F], F16)
            nc.vector.memset(cT, 0.0)
            nc.vector.memset(zero_h, 0.0)

            # ---- one recurrence step ----
            # G layout: [P, mi(16), Bc]; mi = 4*j + {i,f,o,g}
            def step_mms(xg_s, h_lo, h_hi):
                """Emit the step's matmuls. h_lo/h_hi: [P, 2, Bc] fp16 views
                (j=0,1 and j=2,3 of the previous step's h2)."""
                Gt = psp.tile([P, NM * Bc], F32, tag="gates")
                nc.tensor.matmul(Gt, ident, xg_s, start=True, stop=False,
                                 skip_group_check=True)
                for j in range(2):
                    for mi in range(NM):
                        nc.tensor.matmul(
                            Gt[:, mi * Bc:(mi + 1) * Bc],
                            whh[:, (j * NM + mi) * P:(j * NM + mi + 1) * P],
                            h_lo[:, j, :],
                            start=False, stop=False, skip_group_check=True)
                for mh in range(2):
                    for mi in range(mh * 8, mh * 8 + 8):
                        for j in range(2, 4):
                            nc.tensor.matmul(
                                Gt[:, mi * Bc:(mi + 1) * Bc],
                                whh[:, (j * NM + mi) * P:(j * NM + mi + 1) * P],
                                h_hi[:, j - 2, :],
                                start=False, stop=(j == 3), skip_group_check=True)
                return Gt

            def chain_half(Gt, mh, h_new):
                """Elementwise chain for half mh (j in {2mh, 2mh+1}).
                h_new: [P, 2, Bc] fp16 view to write h2 = h/2 into."""
                Gh = Gt[:, mh * 8 * Bc:(mh + 1) * 8 * Bc].rearrange(
                    "p (j g b) -> p j g b", j=2, g=4)
                Sg = actp.tile([P, 2, 4, Bc], F32, tag=f"S{mh}")
                nc.scalar.activation(Sg, Gh, SIG)
                cs = cT[:, 2 * mh:2 * mh + 2, :]
                m = small.tile([P, 2, Bc], F32, tag=f"m{mh}")
                nc.vector.tensor_mul(m, Sg[:, :, 1, :], cs)              # f*c
                t = small.tile([P, 2, Bc], F32, tag=f"t{mh}")
                nc.vector.scalar_tensor_tensor(                          # (sg_g-.5)*i
                    out=t, in0=Sg[:, :, 3, :], scalar=-0.5, in1=Sg[:, :, 0, :],
                    op0=ALU.add, op1=ALU.mult)
                nc.vector.scalar_tensor_tensor(                          # c = 2t + m
                    out=cs, in0=t, scalar=2.0, in1=m,
                    op0=ALU.mult, op1=ALU.add)
                Sc = small.tile([P, 2, Bc], F32, tag=f"sc{mh}")
                nc.scalar.activation(Sc, cs, SIG, scale=2.0)             # sigmoid(2c)
                nc.vector.scalar_tensor_tensor(                          # h2=(Sc-.5)*o
                    out=h_new, in0=Sc, scalar=-0.5, in1=Sg[:, :, 2, :],
                    op0=ALU.add, op1=ALU.mult)

            hb_prev = [None]
            xs_t, o1_t, xgc_t = {}, {}, {}

            def load_inputs(kk):
                if kk >= iters:
                    return
                xs_t[kk] = xsp.tile([P, CHUNK * Bc], F16, tag="xs", name=f"xs{kk}")
                nc.sync.dma_start(
                    out=xs_t[kk],
                    in_=xm_in[:, kk * CHUNK * Bc:(kk + 1) * CHUNK * Bc])
                if kk >= LAG:
                    o1_t[kk] = o1p.tile([P, CHUNK, F], F16, tag="o1", name=f"o1_{kk}")
                    # issue the gathered-chunk load from the Pool queue: the
                    # trigger waits on the AllGather semaphore, and on the ACT
                    # queue that wait parks ahead of the chain's sigmoids and
                    # stalls them whenever a gather runs late.  Pool runs the
                    # AGs themselves, so the wait there is naturally ordered
                    # and blocks nothing.
                    nc.gpsimd.dma_start(
                        out=o1_t[kk],
                        in_=out1_full[kk - LAG][bass.ds(slot, 1)].rearrange(
                            "o p sf -> p (o sf)").rearrange(
                            "p (s f) -> p s f", s=CHUNK))

            def prod_mi(kk, mi, dep_rhs):
                """Produce xg chunk kk for m-block mi (all CHUNK steps at once).

                xg = wib@x_kk + wob@out1_{kk-LAG} + bias; exactly one of
                wib/wob is nonzero per core.  bias0 ensures phantom chunks
                (kk < LAG on L2 cores) keep h,c exactly zero.

                The first matmul adds an exact zero (zero1.T @ dep_rhs) but
                reads the current step's h tile: it pins the sweep to this
                step so the static scheduler cannot hoist all production to
                the chunk boundary (which left the recurrence bursts without
                PE filler work -> HAM-throttled cold matmuls mid-chunk).
                """
                if kk >= iters:
                    return
                xsv = xs_t[kk].rearrange("p (s b) -> p s b", s=CHUNK)
                pp = ppp.tile([P, CHUNK * Bc], F32, tag="pp")
                # ordering-only matmul: writes zeros to pp[:, :F]; the wib
                # matmul below (start=True over the full tile) overwrites
                # them, so the value contribution is nil -- only the FIFO
                # position matters.
                nc.tensor.matmul(
                    pp[:, 0:F], zero1, dep_rhs,
                    start=True, stop=True, skip_group_check=True)
                nc.tensor.matmul(
                    pp, wib[:, mi * P:(mi + 1) * P], xsv,
                    start=True, stop=(kk < LAG), skip_group_check=True)
                if kk >= LAG:
                    for j in range(NJ):
                        nc.tensor.matmul(
                            pp,
                            wob[:, (j * NM + mi) * P:(j * NM + mi + 1) * P],
                            o1_t[kk][:, :, j * Bc:(j + 1) * Bc],
                            start=False, stop=(j == NJ - 1), skip_group_check=True)
                return pp

            def prod_bias(kk, mi, pp):
                """Bias add for a production sweep.  Emitted AFTER the step's
                chain so the 512-col DVE op queues behind the critical-path
                chain ops instead of head-of-line blocking them."""
                if kk >= iters:
                    return
                nc.vector.tensor_scalar_add(
                    xgc_t[kk][:, :, mi, :],
                    pp.rearrange("p (s b) -> p s b", s=CHUNK),
                    (bias0 if kk < LAG else bias)[:, mi:mi + 1])

            # ---- prologue: chunk 0 inputs + production ----
            load_inputs(0)
            xgc_t[0] = xgp.tile([P, CHUNK, NM, Bc], F16, tag="xg", name="xg0")
            for mi in range(NM):
                pp0 = prod_mi(0, mi, whh[0:1, 0:F])
                prod_bias(0, mi, pp0)

            # ---- pipeline over iterations ----
            for k in range(iters):
                load_inputs(k + 1)
                if k + 1 < iters:
                    xgc_t[k + 1] = xgp.tile([P, CHUNK, NM, Bc], F16, tag="xg",
                                            name=f"xg{k+1}")
                xgc = xgc_t[k]
                for u in range(CHUNK // SUB_H):
                    hb = hrol.tile([P, SUB_H, NJ, Bc], F16, tag="h")
                    for s in range(SUB_H):
                        st = u * SUB_H + s
                        if hb_prev[0] is None and st == 0:
                            hp = zero_h[:, :].rearrange("p (j b) -> p j b", j=NJ)
                        elif s == 0:
                            hp = hb_prev[0][:, SUB_H - 1, :, :]
                        else:
                            hp = hb[:, s - 1, :, :]
                        Gt = step_mms(xgc[:, st, :, :], hp[:, 0:2, :], hp[:, 2:4, :])
                        # one production m-block per step fills the PE gap
                        # while this step's chain runs; gating it on the
                        # PREVIOUS h makes it ready exactly when this step's
                        # recurrence is, so it queues right behind it and
                        # cannot be hoisted to the chunk boundary.
                        ppx = prod_mi(k + 1, st,
                                      hp[0:1, :, :].rearrange("p j b -> p (j b)"))
                        chain_half(Gt, 0, hb[:, s, 0:2, :])
                        chain_half(Gt, 1, hb[:, s, 2:4, :])
                        prod_bias(k + 1, st, ppx)
                    hb_prev[0] = hb
                    nc.sync.dma_start(
                        out=out1_stage[k][:, u * SUB_H * F:(u + 1) * SUB_H * F].rearrange(
                            "p (s f) -> p s f", s=SUB_H),
                        in_=hb.rearrange("p s j b -> p s (j b)"))
                nc.gpsimd.collective_compute(
                    "AllGather", mybir.AluOpType.bypass,
                    replica_groups=[CORE_IDS],
                    ins=[out1_stage[k][:, :]], outs=[out1_full[k][:, :, :]])
                xs_t.pop(k, None)
                o1_t.pop(k, None)
                xgc_t.pop(k, None)

            # ---- final outputs (h2 scaled to h on the host) ----
            h32 = state.tile([P, F], F32)
            nc.vector.tensor_copy(
                h32, hb_prev[0][:, SUB_H - 1, :, :].rearrange("p j b -> p (j b)"))
            nc.sync.dma_start(out=h_out[:, :], in_=h32)
            nc.sync.dma_start(
                out=c_out[:, :], in_=cT.rearrange("p j b -> p (j b)"))
    return nc


# ---------------- host-side packing ----------------

def _perm_rows():
    gate_base = {"i": 0, "f": H, "g": 2 * H, "o": 3 * H}
    order = []
    for j in range(NJ):
        for gname in ("i", "f", "o", "g"):
            start = gate_base[gname] + j * P
            order.extend(range(start, start + P))
    return np.array(order)


def _g_row_mask():
    """Mask (in permuted row order) of the g-gate rows (gidx==3)."""
    m = np.zeros(G4, bool)
    m[(np.arange(G4) // P) % 4 == 3] = True
    return m


def _pack_whh(W):
    Wr = W.reshape(NM, P, NJ, P)       # [mi, q, j, p]
    out = Wr.transpose(3, 2, 0, 1)     # [p, j, mi, q]
    return np.ascontiguousarray(out.reshape(P, NJ * NM * P)).astype(np.float16)


def pack_inputs(x, W_ih1, W_hh1, b_ih1, b_hh1, W_ih2, W_hh2, b_ih2, b_hh2, nch):
    iters = nch + LAG
    perm = _perm_rows()
    gmask = _g_row_mask()
    W_ih1 = np.asarray(W_ih1, np.float64)[perm]
    W_hh1 = np.asarray(W_hh1, np.float64)[perm]
    W_ih2 = np.asarray(W_ih2, np.float64)[perm]
    W_hh2 = np.asarray(W_hh2, np.float64)[perm]
    bias1 = (np.asarray(b_ih1, np.float64) + np.asarray(b_hh1, np.float64))[perm]
    bias2 = (np.asarray(b_ih2, np.float64) + np.asarray(b_hh2, np.float64))[perm]

    # tanh->sigmoid trick: double the g-gate rows of every pre-activation
    # contribution.
    for a in (W_ih1, W_hh1, W_ih2, W_hh2, bias1, bias2):
        a[gmask] *= 2.0
    # h2 = h/2 compensation: double the columns that consume h
    # (W_hh1/W_hh2 all columns, W_ih2 all columns; x is not h -> W_ih1 no).
    W_hh1 *= 2.0
    W_hh2 *= 2.0
    W_ih2 *= 2.0

    bias1 = bias1.astype(np.float32)
    bias2 = bias2.astype(np.float32)
    whh1 = _pack_whh(W_hh1)
    whh2 = _pack_whh(W_hh2)
    wob = _pack_whh(W_ih2)
    wib = np.ascontiguousarray(
        W_ih1.reshape(NM, P, D).transpose(2, 0, 1).reshape(D, NM * P)).astype(np.float16)

    b1p = np.ascontiguousarray(bias1.reshape(NM, P).T)
    b2p = np.ascontiguousarray(bias2.reshape(NM, P).T)
    bias0_l2 = np.full((P, NM), -30.0, np.float32)
    bias0_l2[:, 3::4] = -60.0  # g rows doubled
    ident = np.eye(P, dtype=np.float16)
    zeros_like = lambda a: np.zeros_like(a)

    x16 = np.asarray(x).astype(np.float16)
    T_steps = nch * CHUNK
    in_maps = []
    for c in range(N_CORES):
        q = c % 4
        xq = x16[q * Bc:(q + 1) * Bc, :T_steps, :]          # [Bc, T, D]
        xm = np.zeros((D, iters * CHUNK * Bc), np.float16)
        if c < 4:
            xm[:, :T_steps * Bc] = xq.transpose(2, 1, 0).reshape(D, T_steps * Bc)
        is_l1 = c < 4
        in_maps.append({
            "x_my": xm,
            "whh": whh1 if is_l1 else whh2,
            "wib": wib if is_l1 else zeros_like(wib),
            "wob": zeros_like(wob) if is_l1 else wob,
            "bias": b1p if is_l1 else b2p,
            "bias0": b1p if is_l1 else bias0_l2,
            "ident": ident,
            "roles": np.array([[1 if is_l1 else 0, 0 if is_l1 else 1, q]], np.uint32),
        })
    return in_maps


def unpack_outputs(results):
    hs, cs = [], []
    for q in range(4):
        for name, acc, scale in (("h_out", hs, 2.0), ("c_out", cs, 1.0)):
            a = results[4 + q][name] * scale
            acc.append(np.ascontiguousarray(
                a.reshape(P, NJ, Bc).transpose(2, 1, 0).reshape(Bc, H)))
    return np.concatenate(hs)[None], np.concatenate(cs)[None]


# ---------------- harness entry point ----------------

N_CHUNKS_FULL = NCH_FULL = 32  # T = 512 (nch chunks of CHUNK=16)

_CACHE = {}


def _get_nc():
    if "nc" not in _CACHE:
        nc = build(NCH_FULL)
        nc.finalize()
        _CACHE["nc"] = nc
    return _CACHE["nc"]


def kernel(x, W_ih1, W_hh1, b_ih1, b_hh1, W_ih2, W_hh2, b_ih2, b_hh2):
    """Full (unsharded) inputs -> (h_T [1, B, H], c_T [1, B, H]) fp32."""
    import time as _time
    from concourse.bass_utils import run_bass_kernel_spmd
    nc = _get_nc()
    in_maps = pack_inputs(x, W_ih1, W_hh1, b_ih1, b_hh1,
                          W_ih2, W_hh2, b_ih2, b_hh2, NCH_FULL)
    last_err = None
    for attempt in range(3):
        try:
            res = run_bass_kernel_spmd(nc, in_maps, CORE_IDS)
            h, c = unpack_outputs(res.results)
            return h.astype(np.float32), c.astype(np.float32)
        except Exception as e:  # transient device wedge: back off and retry
            last_err = e
            _time.sleep(5 * (attempt + 1))
    raise last_err


if __name__ == "__main__":
    # CoreSim verification at small T
    nch = int(sys.argv[1]) if len(sys.argv) > 1 else 2
    T_steps = nch * CHUNK
    rng = np.random.default_rng(0)
    s1 = 1.0 / np.sqrt(H)
    x = rng.standard_normal((B, T_steps, D), dtype=np.float32)
    W_ih1 = rng.uniform(-s1, s1, (G4, D)).astype(np.float32)
    W_hh1 = rng.uniform(-s1, s1, (G4, H)).astype(np.float32)
    b_ih1 = rng.uniform(-s1, s1, G4).astype(np.float32)
    b_hh1 = rng.uniform(-s1, s1, G4).astype(np.float32)
    W_ih2 = rng.uniform(-s1, s1, (G4, H)).astype(np.float32)
    W_hh2 = rng.uniform(-s1, s1, (G4, H)).astype(np.float32)
    b_ih2 = rng.uniform(-s1, s1, G4).astype(np.float32)
    b_hh2 = rng.uniform(-s1, s1, G4).astype(np.float32)

    def np_lstm(x, W_ih, W_hh, b):
        Bs, T, _ = x.shape
        Hn = W_hh.shape[1]
        h = np.zeros((Bs, Hn), np.float32)
        c = np.zeros((Bs, Hn), np.float32)
        outs = np.zeros((Bs, T, Hn), np.float32)
        xg = x @ W_ih.T + b
        sig = lambda v: 1.0 / (1.0 + np.exp(-v))
        for t in range(T):
            g = xg[:, t] + h @ W_hh.T
            i, f, gg, o = np.split(g, 4, axis=-1)
            i, f, o = sig(i), sig(f), sig(o)
            gg = np.tanh(gg)
            c = f * c + i * gg
            h = o * np.tanh(c)
            outs[:, t] = h
        return outs, h, c

    o1, _, _ = np_lstm(x, W_ih1, W_hh1, b_ih1 + b_hh1)
    _, h_ref, c_ref = np_lstm(o1, W_ih2, W_hh2, b_ih2 + b_hh2)

    nc = build(nch)
    nc.finalize()
    in_maps = pack_inputs(x, W_ih1, W_hh1, b_ih1, b_hh1,
                          W_ih2, W_hh2, b_ih2, b_hh2, nch)

    from concourse.bass_interp import MultiCoreSim
    sim = MultiCoreSim(nc, num_cores=N_CORES, require_finite=False,
                       require_nnan=False)
    for cid, core_sim in sim.cores.items():
        for name, val in in_maps[cid].items():
            core_sim.tensor(name)[:] = val
    sim.simulate()
    results = [{n: np.asarray(sim.cores[c].tensor(n)) for n in ("h_out", "c_out")}
               for c in range(N_CORES)]
    h_got, c_got = unpack_outputs(results)
    for name, got, exp in (("h", h_got[0], h_ref), ("c", c_got[0], c_ref)):
        err = np.abs(got - exp).max()
        scale = np.abs(exp).max()
        print(f"{name}: absmax={err:.3e} scale={scale:.3f} rel={err/scale:.3e}")
